# revision 13
# baseline (speedup 1.0000x reference)
"""Trainium2 Bass kernel for nn_Block_22720376995910 (attention + dense-MoE block).

Sharding: token-parallel across 8 cores. Core c owns 512 tokens (batch c//2,
half c%2). Every core runs all 16 experts on its own 512 tokens; attention is
computed per-core for its q rows against the full causal prefix of its batch.
Causality differences between cores live in host-supplied additive mask
values, so the SPMD program is identical on all cores. The host concatenates
the 8 token slices into the full output.
"""
import os
import numpy as np
import ml_dtypes

import concourse.bass as bass
import concourse.mybir as mybir
import concourse.tile as tile
from concourse.vector_clock import ScopedClock
import bass_rust

F32 = mybir.dt.float32
BF16 = mybir.dt.bfloat16
AFT = mybir.ActivationFunctionType
ALU = mybir.AluOpType
AX = mybir.AxisListType

B, T, C = 4, 1024, 768
H, HD = 12, 64
E, DFF = 16, 3072
EPS = 1e-5

TOWN = 512              # tokens owned per core
NCB = C // 128          # 6 c-blocks
NTT_OWN = TOWN // 128   # 4 own token tiles
NTT_PRE = T // 128      # 8 prefix token tiles
NDB = DFF // 128        # 24 dff blocks
NFB_KV = (2 * C) // 128 # 12 kv feature blocks
NFB_Q = C // 128        # 6 q feature blocks

N_HEADS = int(os.environ.get("KB_HEADS", H))
N_EXPERTS = int(os.environ.get("KB_EXPERTS", E))


# ---------------------------------------------------------------------------
# walrus workaround: this walrus build accepts at most one embedded sem-wait
# on an SP Drain, but TileContext._drain_and_barrier attaches one wait per
# touched DMA lane to a single drain. Split them, one wait per drain.
def _drain_and_barrier_split(self, tick_clock, wait_clock):
    d0 = self.nc.sync.drain()
    wait_clock.add_sem_waits(d0.ins, ScopedClock({None: tick_clock.global_clock}))
    si = d0.ins.sync_info
    waits = list(si.on_wait) if si and si.on_wait else []
    if len(waits) > 1:
        si.on_wait = waits[:1]
        for wi in waits[1:]:
            di = self.nc.sync.drain()
            di.ins.sync_info = bass_rust.SyncInfo(on_wait=[wi], on_update=[])
    self.nc.all_engine_barrier()
    assert self.sems is not None
    popped = self.nc._tile_sem_poison_stack.pop()
    assert popped is self._sem_poison
    self.nc.clear_and_free_semaphores(list(self.sems.allocated().values()))
    self.nc.all_engine_barrier()


tile.TileContext._drain_and_barrier = _drain_and_barrier_split


def _split_multi_waits(nc, limit=1):
    """This walrus build accepts at most one embedded sem-wait per
    instruction. Hoist excess waits onto preceding same-engine NOPs."""
    n_split = 0
    for fn in nc.m.functions:
        for blk in fn.blocks:
            out = []
            for inst in blk.instructions:
                si = getattr(inst, "sync_info", None)
                w = list(si.on_wait) if si and si.on_wait else []
                if len(w) > limit:
                    for j, wi in enumerate(w[: len(w) - limit]):
                        nop = mybir.InstNoOp(
                            name=f"{inst.name}-wsplit{j}", ins=[], outs=[]
                        )
                        nop.engine = inst.engine
                        nop.sync_info = bass_rust.SyncInfo(
                            on_wait=[wi], on_update=[]
                        )
                        out.append(nop)
                        n_split += 1
                    si.on_wait = w[len(w) - limit :]
                out.append(inst)
            blk.instructions = out
    return n_split
# ---------------------------------------------------------------------------


def _ln_tile(nc, pool, x_tile, mu_sl, rstd_sl, z_out, lnw_b):
    """Token-major layernorm of x_tile ([128, C] f32 AP). Writes per-token
    stats into mu_sl/rstd_sl ([128,1] APs) and z = (x-mu)*rstd*lnw into z_out
    (no +ln_b; that's folded downstream)."""
    s = pool.tile([128, 1], F32, tag="ln_s")
    nc.vector.reduce_sum(s[:], x_tile, axis=AX.X)
    nc.vector.tensor_scalar_mul(mu_sl, s[:], 1.0 / C)
    xc = pool.tile([128, C], F32, tag="ln_xc")
    nc.vector.tensor_scalar(xc[:], x_tile, mu_sl, None, op0=ALU.subtract)
    sq = pool.tile([128, C], F32, tag="ln_sq")
    vs = pool.tile([128, 1], F32, tag="ln_vs")
    nc.vector.scalar_tensor_tensor(
        sq[:], xc[:], 1.0, xc[:], op0=ALU.bypass, op1=ALU.mult, accum_out=vs[:]
    )
    v2 = pool.tile([128, 1], F32, tag="ln_v2")
    nc.vector.tensor_scalar(v2[:], vs[:], 1.0 / C, EPS, op0=ALU.mult, op1=ALU.add)
    nc.scalar.sqrt(v2[:], v2[:])
    nc.vector.reciprocal(rstd_sl, v2[:])
    nc.vector.scalar_tensor_tensor(
        z_out, xc[:], rstd_sl, lnw_b, op0=ALU.mult, op1=ALU.mult
    )


def build_program():
    nc = bass.Bass()

    d_xpre = nc.dram_tensor("x_pre", [T, C], F32, kind="ExternalInput")
    d_xown = nc.dram_tensor("x_own", [TOWN, C], F32, kind="ExternalInput")
    d_wq = nc.dram_tensor("wq", [C, C], BF16, kind="ExternalInput")
    d_wkv = nc.dram_tensor("wkv", [C, 2 * C], BF16, kind="ExternalInput")
    d_bq = nc.dram_tensor("bq", [C, 1], F32, kind="ExternalInput")
    d_bkv = nc.dram_tensor("bkv", [2 * C, 1], F32, kind="ExternalInput")
    d_wproj = nc.dram_tensor("wproj", [C, C], BF16, kind="ExternalInput")
    d_bproj = nc.dram_tensor("bproj", [1, C], F32, kind="ExternalInput")
    d_ln1w = nc.dram_tensor("ln1w", [1, C], F32, kind="ExternalInput")
    d_ln2w = nc.dram_tensor("ln2w", [1, C], F32, kind="ExternalInput")
    d_wrout = nc.dram_tensor("wrout", [C, E], F32, kind="ExternalInput")
    d_lbias = nc.dram_tensor("lbias", [1, E], F32, kind="ExternalInput")
    d_w1 = nc.dram_tensor("w1", [E, C, DFF], BF16, kind="ExternalInput")
    d_b1 = nc.dram_tensor("b1t", [128, E, NDB], F32, kind="ExternalInput")
    d_w2 = nc.dram_tensor("w2", [E, DFF, C], BF16, kind="ExternalInput")
    d_bias2 = nc.dram_tensor("bias2", [E + 1, C], F32, kind="ExternalInput")
    d_mask = nc.dram_tensor("mask", [NTT_OWN, 128, T], F32, kind="ExternalInput")
    d_idn = nc.dram_tensor("idn", [128, 128], BF16, kind="ExternalInput")
    d_idn32 = nc.dram_tensor("idn32", [128, 128], F32, kind="ExternalInput")
    d_idndbl = nc.dram_tensor("idn_dbl", [128, 64], BF16, kind="ExternalInput")
    d_sel = nc.dram_tensor("sel", [E + 1, E], F32, kind="ExternalInput")
    d_out = nc.dram_tensor("out_own", [TOWN, C], F32, kind="ExternalOutput")

    with tile.TileContext(nc) as tc:
        with tc.tile_pool(name="persist", bufs=1) as pp:
            # --- persistent across phases ---
            ones1 = pp.tile([1, 128], F32)
            nc.vector.memset(ones1[:], 1.0)
            ln2w_b = pp.tile([128, C], F32)
            x_own = pp.tile([128, NTT_OWN, C], F32)
            nc.gpsimd.dma_start(x_own[:], d_xown.rearrange("(t p) c -> p t c", p=128))
            x2 = pp.tile([128, NTT_OWN, C], F32)
            yacc = pp.tile([128, NTT_OWN, C], F32)
            mu2 = pp.tile([128, NTT_OWN], F32)
            rstd2 = pp.tile([128, NTT_OWN], F32)
            xn2T_bf = pp.tile([128, NCB, TOWN], BF16)
            gTo = pp.tile([E + 1, TOWN], F32)

            # ================= ATTENTION PHASE ============================
            with (
                tc.tile_pool(name="aconst", bufs=1) as ac,
                tc.tile_pool(name="att_big", bufs=1) as ap,
                tc.tile_pool(name="asc", bufs=2) as asc,
            ):
                idn = ac.tile([128, 128], BF16)
                nc.gpsimd.dma_start(idn[:], d_idn[:])
                idn32 = ac.tile([128, 128], F32)
                nc.gpsimd.dma_start(idn32[:], d_idn32[:])
                idn_dbl = ac.tile([128, 64], BF16)
                nc.gpsimd.dma_start(idn_dbl[:], d_idndbl[:])
                masks = ac.tile([128, NTT_OWN, T], F32)
                nc.gpsimd.dma_start(masks[:], d_mask.rearrange("q p k -> p q k"))
                wrout = ac.tile([128, NCB, E], F32)
                nc.gpsimd.dma_start(
                    wrout[:], d_wrout.rearrange("(cb p) e -> p cb e", p=128)
                )
                lbias = ac.tile([1, E], F32)
                nc.gpsimd.dma_start(lbias[:], d_lbias[:])
                bq = ac.tile([128, NFB_Q, 1], F32)
                nc.gpsimd.dma_start(bq[:], d_bq.rearrange("(fb p) o -> p fb o", p=128))
                bkv = ac.tile([128, NFB_KV, 1], F32)
                nc.gpsimd.dma_start(bkv[:], d_bkv.rearrange("(fb p) o -> p fb o", p=128))

                # broadcast rows -> [128, C] via rank-1 matmuls
                ln1w_b = ac.tile([128, C], F32)
                bproj_b = ac.tile([128, C], F32)
                with tc.tile_pool(name="pbc", bufs=1, space="PSUM") as pbc:
                    for row_d, dst in (
                        (d_ln1w, ln1w_b), (d_ln2w, ln2w_b), (d_bproj, bproj_b),
                    ):
                        r = ac.tile([1, C], F32, tag="rowin")
                        nc.gpsimd.dma_start(r[:], row_d[:])
                        for hf in range(2):
                            ps = pbc.tile([128, 384], F32, tag="bc")
                            nc.tensor.matmul(
                                ps[:], ones1[:], r[:, hf * 384 : (hf + 1) * 384],
                                start=True, stop=True,
                            )
                            nc.scalar.copy(dst[:, hf * 384 : (hf + 1) * 384], ps[:])

                # ---- LN1 + transpose + QKV (scoped) ----------------------
                kvT = ap.tile([128, NFB_KV, T], BF16)
                qT = ap.tile([128, NFB_Q, TOWN], BF16)
                with (
                    tc.tile_pool(name="lq", bufs=1) as lq,
                    tc.tile_pool(name="lnsc", bufs=2) as lnsc,
                ):
                    xn1T = lq.tile([128, NCB, T], BF16)
                    xn1oT = lq.tile([128, NCB, TOWN], BF16)
                    with (
                        tc.tile_pool(name="xpre_p", bufs=1) as xp,
                        tc.tile_pool(name="ptr1", bufs=2, space="PSUM") as ptr1,
                    ):
                        xpre = xp.tile([128, NTT_PRE, C], F32)
                        nc.gpsimd.dma_start(
                            xpre[:], d_xpre.rearrange("(t p) c -> p t c", p=128)
                        )
                        mu1 = xp.tile([128, NTT_PRE + NTT_OWN], F32)
                        rstd1 = xp.tile([128, NTT_PRE + NTT_OWN], F32)
                        for tt in range(NTT_PRE):
                            z = lnsc.tile([128, C], BF16, tag="z1")
                            _ln_tile(
                                nc, lnsc, xpre[:, tt, :], mu1[:, tt : tt + 1],
                                rstd1[:, tt : tt + 1], z[:], ln1w_b[:],
                            )
                            for cb in range(NCB):
                                ps = ptr1.tile([128, 128], BF16, tag="t1")
                                nc.tensor.transpose(
                                    ps[:], z[:, cb * 128 : (cb + 1) * 128], idn[:]
                                )
                                nc.scalar.copy(
                                    xn1T[:, cb, tt * 128 : (tt + 1) * 128], ps[:]
                                )
                        for tt in range(NTT_OWN):
                            z = lnsc.tile([128, C], BF16, tag="z1")
                            _ln_tile(
                                nc, lnsc, x_own[:, tt, :],
                                mu1[:, NTT_PRE + tt : NTT_PRE + tt + 1],
                                rstd1[:, NTT_PRE + tt : NTT_PRE + tt + 1], z[:],
                                ln1w_b[:],
                            )
                            for cb in range(NCB):
                                ps = ptr1.tile([128, 128], BF16, tag="t1")
                                nc.tensor.transpose(
                                    ps[:], z[:, cb * 128 : (cb + 1) * 128], idn[:]
                                )
                                nc.scalar.copy(
                                    xn1oT[:, cb, tt * 128 : (tt + 1) * 128], ps[:]
                                )

                    # ---- QKV ------------------------------------------------
                    with (
                        tc.tile_pool(name="wqkv_p", bufs=1) as wp,
                        tc.tile_pool(name="pqkv", bufs=2, space="PSUM") as pqkv,
                    ):
                        for kv_half in range(2):
                            wh = wp.tile([128, NCB, C], BF16, tag="wkv")
                            nc.gpsimd.dma_start(
                                wh[:],
                                d_wkv[:, kv_half * C : (kv_half + 1) * C].rearrange(
                                    "(cb p) f -> p cb f", p=128
                                ),
                            )
                            for fbl in range(NFB_Q):
                                fb = kv_half * NFB_Q + fbl
                                for ch in range(2):
                                    ps = pqkv.tile([128, 512], F32, tag="qkv")
                                    for cb in range(NCB):
                                        nc.tensor.matmul(
                                            ps[:],
                                            wh[:, cb, fbl * 128 : (fbl + 1) * 128],
                                            xn1T[:, cb, ch * 512 : (ch + 1) * 512],
                                            start=(cb == 0), stop=(cb == NCB - 1),
                                        )
                                    nc.scalar.activation(
                                        kvT[:, fb, ch * 512 : (ch + 1) * 512], ps[:],
                                        AFT.Identity, bias=bkv[:, fb, :],
                                    )
                        wh = wp.tile([128, NCB, C], BF16, tag="wkv")
                        nc.gpsimd.dma_start(
                            wh[:], d_wq.rearrange("(cb p) f -> p cb f", p=128)
                        )
                        for fb in range(NFB_Q):
                            ps = pqkv.tile([128, 512], F32, tag="qkv")
                            for cb in range(NCB):
                                nc.tensor.matmul(
                                    ps[:],
                                    wh[:, cb, fb * 128 : (fb + 1) * 128],
                                    xn1oT[:, cb, :],
                                    start=(cb == 0), stop=(cb == NCB - 1),
                                )
                            nc.scalar.activation(
                                qT[:, fb, :], ps[:], AFT.Identity, bias=bq[:, fb, :],
                            )

                # ---- per-head attention ---------------------------------
                yT = ap.tile([128, NCB, TOWN], BF16)
                with (
                    tc.tile_pool(name="ps_s", bufs=2, space="PSUM") as ps_s,
                    tc.tile_pool(name="ps_tr", bufs=1, space="PSUM") as ps_tr,
                    tc.tile_pool(name="ps_v", bufs=1, space="PSUM") as ps_v,
                    tc.tile_pool(name="ps_yt", bufs=2, space="PSUM") as ps_yt,
                    tc.tile_pool(name="att_h", bufs=2) as ahp,
                ):
                    for hp in range(N_HEADS // 2):
                        psy = ps_yt.tile([128, TOWN], F32, tag="yt")
                        for sub in range(2):
                            h = 2 * hp + sub
                            po = (h % 2) * 64
                            fb = h // 2
                            vtok = ahp.tile([128, NTT_PRE, 64], BF16, tag="vtok")
                            psv = ps_v.tile([128, NTT_PRE, 64], BF16, tag="v")
                            for kt in range(NTT_PRE):
                                nc.tensor.transpose(
                                    psv[:, kt, :],
                                    kvT[po : po + 64, NFB_Q + fb,
                                        kt * 128 : (kt + 1) * 128],
                                    idn_dbl[po : po + 64, :],
                                )
                            nc.scalar.copy(vtok[:], psv[:])
                            attT = ahp.tile([128, NTT_PRE, TOWN], BF16, tag="attT")
                            for qt in range(NTT_OWN):
                                pss = ps_s.tile([128, T], F32, tag="s")
                                for ch in range(2):
                                    nc.tensor.matmul(
                                        pss[:, ch * 512 : (ch + 1) * 512],
                                        qT[po : po + 64, fb, qt * 128 : (qt + 1) * 128],
                                        kvT[po : po + 64, fb, ch * 512 : (ch + 1) * 512],
                                        start=True, stop=True,
                                    )
                                sm = ahp.tile([128, T], F32, tag="sm")
                                nc.vector.scalar_tensor_tensor(
                                    sm[:], pss[:], 1.0, masks[:, qt, :],
                                    op0=ALU.bypass, op1=ALU.add,
                                )
                                ex = ahp.tile([128, T], BF16, tag="ex")
                                rs = ahp.tile([128, 1], F32, tag="rs")
                                nc.scalar.activation(
                                    ex[:], sm[:], AFT.Exp, scale=0.125, accum_out=rs[:]
                                )
                                ri = ahp.tile([128, 1], F32, tag="ri")
                                nc.vector.reciprocal(ri[:], rs[:])
                                at = ahp.tile([128, T], BF16, tag="at")
                                nc.vector.tensor_scalar_mul(at[:], ex[:], ri[:])
                                pst = ps_tr.tile([128, NTT_PRE, 128], BF16, tag="tr")
                                for kt in range(NTT_PRE):
                                    nc.tensor.transpose(
                                        pst[:, kt, :],
                                        at[:, kt * 128 : (kt + 1) * 128],
                                        idn[:],
                                    )
                                nc.scalar.copy(
                                    attT[:, :, qt * 128 : (qt + 1) * 128], pst[:]
                                )
                            for kt in range(NTT_PRE):
                                nc.tensor.matmul(
                                    psy[po : po + 64, :],
                                    vtok[:, kt, :],
                                    attT[:, kt, :],
                                    start=(kt == 0), stop=(kt == NTT_PRE - 1),
                                    tile_position=(0, po),
                                )
                        nc.scalar.copy(yT[:, hp, :], psy[:])

                # ---- proj + residual ------------------------------------
                with (
                    tc.tile_pool(name="wproj_p", bufs=1) as wpp,
                    tc.tile_pool(name="ppr", bufs=2, space="PSUM") as ppr,
                ):
                    wproj_t = wpp.tile([128, NCB, C], BF16)
                    nc.gpsimd.dma_start(
                        wproj_t[:], d_wproj.rearrange("(fb p) c -> p fb c", p=128)
                    )
                    for tt in range(NTT_OWN):
                        ps = ppr.tile([128, C], F32, tag="pr")
                        for fb in range(NCB):
                            for off, width in ((0, 512), (512, 256)):
                                nc.tensor.matmul(
                                    ps[:, off : off + width],
                                    yT[:, fb, tt * 128 : (tt + 1) * 128],
                                    wproj_t[:, fb, off : off + width],
                                    start=(fb == 0), stop=(fb == NCB - 1),
                                )
                        t0 = asc.tile([128, C], F32, tag="prt")
                        nc.vector.scalar_tensor_tensor(
                            t0[:], ps[:], 1.0, x_own[:, tt, :],
                            op0=ALU.bypass, op1=ALU.add,
                        )
                        nc.vector.tensor_tensor(
                            x2[:, tt, :], t0[:], bproj_b[:], op=ALU.add
                        )

                # ---- LN2 + f32 transpose + router + gating --------------
                xn2T_f = ap.tile([128, NCB, TOWN], F32)
                with tc.tile_pool(name="ptr2", bufs=2, space="PSUM") as ptr2:
                    for tt in range(NTT_OWN):
                        z2 = asc.tile([128, C], F32, tag="z2")
                        _ln_tile(
                            nc, asc, x2[:, tt, :], mu2[:, tt : tt + 1],
                            rstd2[:, tt : tt + 1], z2[:], ln2w_b[:],
                        )
                        for cb in range(NCB):
                            ps = ptr2.tile([128, 128], F32, tag="t2")
                            nc.tensor.transpose(
                                ps[:], z2[:, cb * 128 : (cb + 1) * 128], idn32[:]
                            )
                            nc.scalar.copy(
                                xn2T_f[:, cb, tt * 128 : (tt + 1) * 128], ps[:]
                            )
                nc.vector.tensor_copy(xn2T_bf[:], xn2T_f[:])

                with (
                    tc.tile_pool(name="prt", bufs=2, space="PSUM") as prt,
                    tc.tile_pool(name="pgt", bufs=2, space="PSUM") as pgt,
                ):
                    for tt in range(NTT_OWN):
                        ps = prt.tile([128, E], F32, tag="lg")
                        nc.tensor.matmul(ps[:], ones1[:], lbias[:], start=True, stop=False)
                        for cb in range(NCB):
                            nc.tensor.matmul(
                                ps[:],
                                xn2T_f[:, cb, tt * 128 : (tt + 1) * 128],
                                wrout[:, cb, :],
                                start=False, stop=(cb == NCB - 1),
                            )
                        pe = asc.tile([128, E], F32, tag="pe")
                        se = asc.tile([128, 1], F32, tag="se")
                        nc.scalar.activation(pe[:], ps[:], AFT.Exp, accum_out=se[:])
                        si = asc.tile([128, 1], F32, tag="si")
                        nc.vector.reciprocal(si[:], se[:])
                        pr = asc.tile([128, E], F32, tag="prb")
                        nc.vector.tensor_scalar_mul(pr[:], pe[:], si[:])
                        m1 = asc.tile([128, 1], F32, tag="m1")
                        nc.vector.reduce_max(m1[:], pr[:], axis=AX.X)
                        eq1 = asc.tile([128, E], F32, tag="eq1")
                        nc.vector.tensor_scalar(eq1[:], pr[:], m1[:], None, op0=ALU.is_ge)
                        p2 = asc.tile([128, E], F32, tag="p2")
                        nc.vector.scalar_tensor_tensor(
                            p2[:], eq1[:], -1e9, pr[:], op0=ALU.mult, op1=ALU.add
                        )
                        m2 = asc.tile([128, 1], F32, tag="m2")
                        nc.vector.reduce_max(m2[:], p2[:], axis=AX.X)
                        sel = asc.tile([128, E], F32, tag="sel")
                        nc.vector.tensor_scalar(sel[:], pr[:], m2[:], None, op0=ALU.is_ge)
                        gt = asc.tile([128, E + 1], F32, tag="gt")
                        nc.vector.tensor_tensor(
                            gt[:, 0:E], pr[:], sel[:], op=ALU.mult
                        )
                        nc.vector.memset(gt[:, E : E + 1], 1.0)
                        pg = pgt.tile([E + 1, 128], F32, tag="gT")
                        nc.tensor.transpose(pg[:], gt[:], idn32[:])
                        nc.scalar.copy(gTo[:, tt * 128 : (tt + 1) * 128], pg[:])

            # ================= MoE PHASE ==================================
            with (
                tc.tile_pool(name="mconst", bufs=1) as mc,
                tc.tile_pool(name="w1p", bufs=1) as w1p,
                tc.tile_pool(name="w2p", bufs=1) as w2p,
                tc.tile_pool(name="htp", bufs=1) as htp,
                tc.tile_pool(name="hsc", bufs=2) as hsc,
                tc.tile_pool(name="gep", bufs=2) as gep,
                tc.tile_pool(name="ph", bufs=2, space="PSUM") as ph,
                tc.tile_pool(name="py", bufs=2, space="PSUM") as py,
                tc.tile_pool(name="pgb", bufs=1, space="PSUM") as pgb,
            ):
                b1t = mc.tile([128, E, NDB], F32)
                nc.gpsimd.dma_start(b1t[:], d_b1[:])
                bias2 = mc.tile([E + 1, C], F32)
                nc.gpsimd.dma_start(bias2[:], d_bias2[:])
                onehot = mc.tile([E + 1, E], F32)
                nc.gpsimd.dma_start(onehot[:], d_sel[:])
                # init yacc with gated expert biases + ln2_b (K=E+1 matmul)
                for tt in range(NTT_OWN):
                    ps = py.tile([128, C], F32, tag="y")
                    for off, width in ((0, 512), (512, 256)):
                        nc.tensor.matmul(
                            ps[:, off : off + width],
                            gTo[:, tt * 128 : (tt + 1) * 128],
                            bias2[:, off : off + width],
                            start=True, stop=True,
                        )
                    nc.scalar.copy(yacc[:, tt, :], ps[:])

                for e in range(N_EXPERTS):
                    w1t = w1p.tile([128, NCB, DFF], BF16, tag="w1")
                    nc.gpsimd.dma_start(
                        w1t[:], d_w1[e].rearrange("(cb p) d -> p cb d", p=128)
                    )
                    w2t = w2p.tile([128, NDB, C], BF16, tag="w2")
                    nc.gpsimd.dma_start(
                        w2t[:], d_w2[e].rearrange("(db p) c -> p db c", p=128)
                    )
                    psr = pgb.tile([1, TOWN], F32, tag="gr")
                    nc.tensor.matmul(
                        psr[:], onehot[:, e : e + 1], gTo[:, :],
                        start=True, stop=True,
                    )
                    grow = gep.tile([1, TOWN], F32, tag="grow")
                    nc.scalar.copy(grow[:], psr[:])
                    psg = pgb.tile([128, TOWN], F32, tag="g")
                    nc.tensor.matmul(psg[:], ones1[:], grow[:], start=True, stop=True)
                    ge = gep.tile([128, TOWN], F32, tag="ge")
                    nc.scalar.copy(ge[:], psg[:])

                    hT = htp.tile([128, NDB, TOWN], BF16, tag="hT")
                    for db in range(NDB):
                        psh = ph.tile([128, TOWN], F32, tag="h")
                        for cb in range(NCB):
                            nc.tensor.matmul(
                                psh[:],
                                w1t[:, cb, db * 128 : (db + 1) * 128],
                                xn2T_bf[:, cb, :],
                                start=(cb == 0), stop=(cb == NCB - 1),
                            )
                        hs = hsc.tile([128, TOWN], F32, tag="hs")
                        nc.scalar.activation(
                            hs[:], psh[:], AFT.Gelu, bias=b1t[:, e, db : db + 1]
                        )
                        nc.vector.tensor_tensor(
                            hT[:, db, :], hs[:], ge[:], op=ALU.mult
                        )
                    for tt in range(NTT_OWN):
                        psy = py.tile([128, C], F32, tag="y")
                        for db in range(NDB):
                            for off, width in ((0, 512), (512, 256)):
                                nc.tensor.matmul(
                                    psy[:, off : off + width],
                                    hT[:, db, tt * 128 : (tt + 1) * 128],
                                    w2t[:, db, off : off + width],
                                    start=(db == 0), stop=(db == NDB - 1),
                                )
                        nc.vector.tensor_tensor(
                            yacc[:, tt, :], psy[:], yacc[:, tt, :], op=ALU.add
                        )

                # ---- final: out = x2 + xn2 + yacc ------------------------
                for tt in range(NTT_OWN):
                    t1 = hsc.tile([128, C], F32, tag="f1")
                    nc.vector.scalar_tensor_tensor(
                        t1[:], x2[:, tt, :], mu2[:, tt : tt + 1], ln2w_b[:],
                        op0=ALU.subtract, op1=ALU.mult,
                    )
                    nc.vector.tensor_tensor(
                        yacc[:, tt, :], yacc[:, tt, :], x2[:, tt, :], op=ALU.add
                    )
                    ot = hsc.tile([128, C], F32, tag="f3")
                    nc.vector.scalar_tensor_tensor(
                        ot[:], t1[:], rstd2[:, tt : tt + 1], yacc[:, tt, :],
                        op0=ALU.mult, op1=ALU.add,
                    )
                    nc.gpsimd.dma_start(d_out[tt * 128 : (tt + 1) * 128, :], ot[:])
    _split_multi_waits(nc)
    return nc


# ---------------------------------------------------------------------------
# Host-side input prep
# ---------------------------------------------------------------------------
def _bf16(a):
    return np.ascontiguousarray(np.asarray(a, dtype=np.float32)).astype(
        ml_dtypes.bfloat16
    )


def _sel_matrix():
    s = np.zeros((E + 1, E), np.float32)
    for e in range(E):
        s[e, e] = 1.0
    return s


def prep_inputs(inputs):
    x = np.asarray(inputs["x"], np.float32)
    ln1_b = np.asarray(inputs["ln1_b"], np.float64)
    ln2_b = np.asarray(inputs["ln2_b"], np.float64)
    W_attn = np.asarray(inputs["W_attn"], np.float32)
    b_attn = np.asarray(inputs["b_attn"], np.float64)
    W1 = np.asarray(inputs["W1"], np.float32)
    b1 = np.asarray(inputs["b1"], np.float64)

    battn_fold = (b_attn + ln1_b @ W_attn.astype(np.float64)).astype(np.float32)
    b1_fold = (b1 + np.einsum("c,ecd->ed", ln2_b, W1.astype(np.float64))).astype(
        np.float32
    )
    b1t = np.ascontiguousarray(b1_fold.reshape(E, NDB, 128).transpose(2, 0, 1))
    bias2 = np.concatenate(
        [np.asarray(inputs["b2"], np.float32), ln2_b.astype(np.float32)[None, :]],
        axis=0,
    )
    lbias = (ln2_b @ np.asarray(inputs["W_router"], np.float64)).astype(np.float32)[
        None, :
    ]

    idn = np.eye(128, dtype=np.float32)
    idn_dbl = np.concatenate([np.eye(64, dtype=np.float32)] * 2, axis=0)

    common = {
        "wq": _bf16(W_attn[:, :C]),
        "wkv": _bf16(W_attn[:, C:]),
        "bq": battn_fold[:C, None].copy(),
        "bkv": battn_fold[C:, None].copy(),
        "wproj": _bf16(inputs["W_proj"]),
        "bproj": np.asarray(inputs["b_proj"], np.float32)[None, :].copy(),
        "ln1w": np.asarray(inputs["ln1_w"], np.float32)[None, :].copy(),
        "ln2w": np.asarray(inputs["ln2_w"], np.float32)[None, :].copy(),
        "wrout": np.ascontiguousarray(np.asarray(inputs["W_router"], np.float32)),
        "lbias": lbias,
        "w1": _bf16(W1),
        "b1t": b1t,
        "w2": _bf16(inputs["W2"]),
        "bias2": np.ascontiguousarray(bias2),
        "idn": _bf16(idn),
        "idn32": idn,
        "idn_dbl": _bf16(idn_dbl),
        "sel": _sel_matrix(),
    }

    in_maps = []
    for c in range(8):
        b, half = c // 2, c % 2
        q0 = half * TOWN
        qglob = q0 + np.arange(TOWN).reshape(NTT_OWN, 128)
        kk = np.arange(T)
        mask = np.where(
            kk[None, None, :] <= qglob[:, :, None], 0.0, -1e9
        ).astype(np.float32)
        m = dict(common)
        m["x_pre"] = np.ascontiguousarray(x[b])
        m["x_own"] = np.ascontiguousarray(x[b, q0 : q0 + TOWN])
        m["mask"] = np.ascontiguousarray(mask)
        in_maps.append(m)
    return in_maps


_PROGRAM = None


def get_program():
    global _PROGRAM
    if _PROGRAM is None:
        _PROGRAM = build_program()
    return _PROGRAM


def kernel(**inputs):
    from concourse import bass2jax

    nc = get_program()
    in_maps = prep_inputs(inputs)
    results = bass2jax.run_bass_via_pjrt(nc, in_maps, n_cores=8)
    out = np.stack([results[c]["out_own"] for c in range(8)], axis=0)
    return out.reshape(B, T, C)
